# revision 7
# baseline (speedup 1.0000x reference)
"""DigitCaps dynamic-routing kernel for 8 Trainium2 NeuronCores.

Strategy: data-parallel over batch (32 per core), W replicated. u_hat is never
materialized; each routing iteration folds the routing coefficients c_ij into a
bf16 copy of W and computes s via a single 72-chunk PE matmul with contraction
over (i, r). The per-iteration agreement statistic A[r,c] (batch mean of
<u_hat, v>) is computed as A = sum_{i,d} W ⊙ (p^T v) and all-reduced across the
8 cores (the routing logits b_ij are shared across the full batch). The third
iteration's b_ij update is dead code in the reference, so only 2 all-reduces run.
"""
import numpy as np
from contextlib import ExitStack

import concourse.bass as bass
from concourse import bacc
import concourse.tile as tile
from concourse import mybir
from concourse.bass_utils import run_bass_kernel_spmd
from concourse.masks import make_identity

N_CORES = 8
B_FULL, R, C, D, I = 256, 1152, 10, 16, 8
B = B_FULL // N_CORES          # 32 batch per core
G = R // 128                   # 9 chunks of 128 routes
RI = R * I                     # 9216
CD = C * D                     # 160
CDI = C * D * I                # 1280
NUM_IT = 3

FP32 = mybir.dt.float32
BF16 = mybir.dt.bfloat16
ALU = mybir.AluOpType
AX = mybir.AxisListType
AF = mybir.ActivationFunctionType


def _build_body(ctx: ExitStack, tc: "tile.TileContext", p_dram, w_dram, v_dram):
    nc = tc.nc

    consts = ctx.enter_context(tc.tile_pool(name="consts", bufs=1))
    pers = ctx.enter_context(tc.tile_pool(name="pers", bufs=1))
    small = ctx.enter_context(tc.tile_pool(name="small", bufs=2))
    dram = ctx.enter_context(tc.tile_pool(name="dram", bufs=2, space="DRAM"))
    ps_s = ctx.enter_context(tc.tile_pool(name="ps_s", bufs=2, space="PSUM"))
    ps_y = ctx.enter_context(tc.tile_pool(name="ps_y", bufs=3, space="PSUM"))
    ps_t = ctx.enter_context(tc.tile_pool(name="ps_t", bufs=2, space="PSUM"))

    # ---------------- stage 0: loads, bf16 conversion, p transpose ----------------
    pb = pers.tile([B, RI], BF16, tag="pb")
    pb_v = pb[:].rearrange("b (g j i) -> b g j i", g=G, j=128, i=I)
    ident = consts.tile([B, B], BF16, tag="ident")
    make_identity(nc, ident[:])
    pT = pers.tile([128, I * G * B], BF16, tag="pT")
    pT_v = pT[:].rearrange("p (i g b) -> p g i b", i=I, g=G, b=B)
    wre = [pers.tile([128, CDI], BF16, tag=f"wre{g}", name=f"wre{g}") for g in range(G)]

    with ExitStack() as s0:
        # fp32 staging pools: released after stage 0 so their SBUF is reusable
        stage = s0.enter_context(tc.tile_pool(name="stage", bufs=1))
        w32p = s0.enter_context(tc.tile_pool(name="w32p", bufs=G))

        p32 = stage.tile([B, RI], FP32, tag="p32")
        nc.sync.dma_start(p32[:], p_dram[:, :])
        # split the fp32->bf16 convert across DVE and ACT
        half = RI // 2
        nc.vector.tensor_copy(pb[:, :half], p32[:, :half])
        nc.scalar.copy(pb[:, half:], p32[:, half:])

        # pT[:, (i*G+g)*B : +B] = p[b, (r in chunk g, i)]^T  -- [128, 32] blocks
        for g in range(G):
            tp = ps_t.tile([128, I * B], BF16, tag="tp")
            for i in range(I):
                nc.tensor.transpose(tp[:, i * B:(i + 1) * B], pb_v[:, g, :, i],
                                    ident[:])
            tp_v = tp[:].rearrange("p (i b) -> p i b", i=I, b=B)
            nc.scalar.copy(pT_v[:, g], tp_v)

        # W: load fp32 natural [r, (c d i)], convert to bf16 i-major [r, (i c d)]
        for g in range(G):
            w32 = w32p.tile([128, CDI], FP32)
            nc.sync.dma_start(w32[:], w_dram[128 * g:128 * (g + 1), :])
            src = w32[:].rearrange("p (c d i) -> p i c d", c=C, d=D, i=I)
            dst = wre[g][:].rearrange("p (i c d) -> p i c d", i=I, c=C, d=D)
            eng = (nc.vector, nc.scalar, nc.gpsimd)[g % 3]
            if eng is nc.scalar:
                eng.copy(dst, src)
            else:
                eng.tensor_copy(dst, src)

    wcp = ctx.enter_context(tc.tile_pool(name="wcp", bufs=1))
    work = ctx.enter_context(tc.tile_pool(name="work", bufs=2))

    # routing logits, [128, (g c)] layout
    bij = pers.tile([128, G * C], FP32, tag="bij")
    nc.gpsimd.memset(bij[:], 0.0)
    bij_v = bij[:].rearrange("p (g c) -> p g c", g=G, c=C)

    # ---------------- routing iterations ----------------
    for t in range(NUM_IT):
        last = t == NUM_IT - 1
        if t == 0:
            wc = wre                      # c_ij uniform: fold 0.1 into squash
            sqrt_e = 0.1
            e_scale = 0.01
        else:
            sqrt_e = 1.0
            e_scale = 1.0
            # softmax over c of bij -> cbb (bf16)
            mx = small.tile([128, G], FP32, tag="mx")
            nc.vector.tensor_reduce(mx[:], bij_v, axis=AX.X, op=ALU.max)
            eb = small.tile([128, G * C], FP32, tag="eb")
            eb_v = eb[:].rearrange("p (g c) -> p g c", g=G, c=C)
            mxb = mx[:].unsqueeze(2).broadcast_to([128, G, C])
            nc.vector.tensor_tensor(eb_v, bij_v, mxb, op=ALU.subtract)
            nc.scalar.activation(eb[:], eb[:], AF.Exp)
            sm = small.tile([128, G], FP32, tag="sm")
            nc.vector.tensor_reduce(sm[:], eb_v, axis=AX.X, op=ALU.add)
            rc = small.tile([128, G], FP32, tag="rc")
            nc.vector.reciprocal(rc[:], sm[:])
            cbb = small.tile([128, G * C], BF16, tag="cbb")
            cbb_v = cbb[:].rearrange("p (g c) -> p g c", g=G, c=C)
            rcb = rc[:].unsqueeze(2).broadcast_to([128, G, C])
            nc.vector.tensor_tensor(cbb_v, eb_v, rcb, op=ALU.mult)

            # Wc[g] = wre[g] * c  (broadcast over i and d)
            wc = [wcp.tile([128, CDI], BF16, tag=f"wc{g}", name=f"wc{g}_{t}") for g in range(G)]
            for g in range(G):
                w4 = wre[g][:].rearrange("p (i c d) -> p i c d", i=I, c=C, d=D)
                o4 = wc[g][:].rearrange("p (i c d) -> p i c d", i=I, c=C, d=D)
                cb4 = cbb[:, g * C:(g + 1) * C].unsqueeze(1).unsqueeze(3) \
                    .broadcast_to([128, I, C, D])
                eng = nc.vector if g % 2 == 0 else nc.gpsimd
                eng.tensor_tensor(o4, w4, cb4, op=ALU.mult)

        # s[b, (c d)] = sum_{i, r} pT^T @ Wc   (72-chunk PSUM accumulation)
        s_ps = ps_s.tile([B, CD], FP32, tag="s_ps")
        n_mm = 0
        for g in range(G):
            for i in range(I):
                k = i * G + g
                nc.tensor.matmul(
                    s_ps[:],
                    pT[:, k * B:(k + 1) * B],
                    wc[g][:, i * CD:(i + 1) * CD],
                    start=(n_mm == 0),
                    stop=(n_mm == G * I - 1),
                )
                n_mm += 1

        # squash: v = s_eff * sqrt(sq)/(1+sq), sq = |s_eff|^2, s_eff = sqrt_e * s
        s2 = small.tile([B, CD], FP32, tag="s2")
        nc.scalar.square(s2[:], s_ps[:])
        sq = small.tile([B, C], FP32, tag="sq")
        nc.vector.tensor_reduce(sq[:], s2[:].rearrange("b (c d) -> b c d", c=C, d=D),
                                axis=AX.X, op=ALU.add)
        r1 = small.tile([B, C], FP32, tag="r1")
        nc.scalar.activation(r1[:], sq[:], AF.Sqrt, scale=e_scale)
        den = small.tile([B, C], FP32, tag="den")
        nc.vector.tensor_scalar(den[:], sq[:], e_scale, 1.0, op0=ALU.mult, op1=ALU.add)
        rec = small.tile([B, C], FP32, tag="rec")
        nc.vector.reciprocal(rec[:], den[:])
        fac = small.tile([B, C], FP32, tag="fac")
        nc.vector.tensor_tensor(fac[:], r1[:], rec[:], op=ALU.mult)

        v32 = small.tile([B, CD], FP32, tag="v32")
        fb = fac[:].unsqueeze(2).broadcast_to([B, C, D])
        nc.vector.scalar_tensor_tensor(
            out=v32[:].rearrange("b (c d) -> b c d", c=C, d=D),
            in0=s_ps[:].rearrange("b (c d) -> b c d", c=C, d=D),
            scalar=sqrt_e, op0=ALU.mult, in1=fb, op1=ALU.mult)

        if last:
            nc.sync.dma_start(v_dram[:, :], v32[:])
            continue

        # ---- agreement stats: A[r, c] = sum_{i,d} W ⊙ (p^T v), then AllReduce ----
        vb = small.tile([B, CD], BF16, tag="vb")
        nc.vector.tensor_copy(vb[:], v32[:])

        Apart = pers.tile([128, G * C], FP32, tag="Apart")
        for g in range(G):
            y_sb = work.tile([128, CDI], BF16, tag="y_sb")
            for i in range(I):
                y_ps = ps_y.tile([128, CD], FP32, tag="y_ps")
                nc.tensor.matmul(y_ps[:], pb_v[:, g, :, i], vb[:],
                                 start=True, stop=True)
                nc.scalar.copy(y_sb[:, i * CD:(i + 1) * CD], y_ps[:])
            prod = work.tile([128, CDI], BF16, tag="prod")
            eng = nc.vector if g % 2 == 0 else nc.gpsimd
            eng.tensor_tensor(prod[:], wre[g][:], y_sb[:], op=ALU.mult)
            nc.vector.tensor_reduce(
                Apart[:, g * C:(g + 1) * C],
                prod[:].rearrange("p (i c d) -> p c i d", i=I, c=C, d=D),
                axis=AX.XY, op=ALU.add)

        cc_in = dram.tile([128, G * C], FP32, tag="cc_in")
        cc_out = dram.tile([128, G * C], FP32, tag="cc_out")
        nc.sync.dma_start(cc_in[:], Apart[:])
        nc.gpsimd.collective_compute(
            "AllReduce", ALU.add,
            replica_groups=[list(range(N_CORES))],
            ins=[cc_in[:].opt()],
            outs=[cc_out[:].opt()],
        )
        acc = small.tile([128, G * C], FP32, tag="acc")
        nc.sync.dma_start(acc[:], cc_out[:])
        nc.vector.scalar_tensor_tensor(
            out=bij[:], in0=acc[:], scalar=1.0 / B_FULL, op0=ALU.mult,
            in1=bij[:], op1=ALU.add)


_CACHED = None


def _build():
    global _CACHED
    if _CACHED is not None:
        return _CACHED
    nc = bacc.Bacc("TRN2", target_bir_lowering=False, debug=False,
                   num_devices=N_CORES)
    p_dram = nc.dram_tensor("p_in", [B, RI], FP32, kind="ExternalInput").ap()
    w_dram = nc.dram_tensor("w_in", [R, CDI], FP32, kind="ExternalInput").ap()
    v_dram = nc.dram_tensor("v_out", [B, CD], FP32, kind="ExternalOutput").ap()
    with tile.TileContext(nc) as tc:
        with ExitStack() as ctx:
            _build_body(ctx, tc, p_dram, w_dram, v_dram)
    nc.finalize()
    _CACHED = nc
    return nc


def kernel(prim_caps: np.ndarray, W: np.ndarray, _trace: bool = False):
    assert prim_caps.shape == (B_FULL, R, I) and W.shape == (1, R, C, D, I)
    nc = _build()
    p_flat = np.ascontiguousarray(prim_caps.reshape(B_FULL, RI).astype(np.float32))
    w_flat = np.ascontiguousarray(W.reshape(R, CDI).astype(np.float32))
    in_maps = [
        {"p_in": np.ascontiguousarray(p_flat[k * B:(k + 1) * B]), "w_in": w_flat}
        for k in range(N_CORES)
    ]
    res = run_bass_kernel_spmd(nc, in_maps, core_ids=list(range(N_CORES)),
                               trace=_trace)
    out = np.concatenate(
        [res.results[k]["v_out"].reshape(B, C, D, 1) for k in range(N_CORES)],
        axis=0)
    if _trace:
        return out, res
    return out
